# revision 1
# baseline (speedup 1.0000x reference)
"""Expert-parallel sparse MoE kernel for Trainium2 (8 NeuronCores).

Problem: gpt_oss-style top-2-of-8 MoE MLP over T=4096 tokens, H=1024,
I=1024. Sharding: expert-parallel — core c owns expert c's weights.

Host side (this file, numpy): compute router top-2 *indices* (the dispatch
decision), shard tokens to expert-owning cores (the "all-to-all dispatch" is
realized as host-side sharding since full inputs start on the host), and
scatter-add the per-expert outputs back into the full [T, H] output (the
"combine").

Device side (Bass/Tile, per core, SPMD): router logits + top-2 softmax
weights for its gathered tokens (router weight is column-permuted per core so
column 0 is always "own expert" — keeps the graph SPMD), fused gate/up
projection, gpt_oss GLU activation, down projection, scaling by the routing
weight. Big matmuls run in bf16 (PSUM accumulation in f32); routing weights
are computed from the same bf16 logits but selection comes from host f32
logits, so selection is never perturbed.
"""

import time

import numpy as np
import ml_dtypes

import concourse.tile as tile
from concourse import bacc, mybir
from concourse.bass_utils import run_bass_kernel_spmd

# Model dims (hardcoded per problem spec)
B, S, H, E, I, K = 2, 2048, 1024, 8, 1024, 2
ALPHA, LIMIT = 1.702, 7.0
T = B * S
P = 128
HB = H // P  # 8 h-chunks
IB = I // P  # 8 i-chunks
N_CORES = 8

BF16 = mybir.dt.bfloat16
F32 = mybir.dt.float32
NP_BF16 = ml_dtypes.bfloat16


def _ceil_to(x, m):
    return ((x + m - 1) // m) * m


def _chunks(total, step):
    out = []
    o = 0
    while o < total:
        w = min(step, total - o)
        out.append((o, w))
        o += w
    return out


def build_expert_kernel(C: int, has_bias: bool, reps: int = 1,
                        bench: bool = False, resident: bool = False,
                        no_router: bool = False):
    """Build the per-core Bass graph. C = token capacity (multiple of 128).

    reps > 1 replicates the compute body inside one NEFF; bench=True uses
    internal (non-transferred) DRAM for the big tensors; resident=True loads
    weights once outside the rep loop (isolates compute+token-DMA steady
    state). All benchmarking-only knobs."""
    assert C % P == 0
    CB = C // P

    nc = bacc.Bacc("TRN2", target_bir_lowering=False, debug=False,
                   num_devices=N_CORES)

    ikind = dict(kind="Internal") if bench else dict(kind="ExternalInput")
    xg_ap = nc.dram_tensor("xg", [C, H], BF16, **ikind).ap()
    wg_ap = nc.dram_tensor("wg", [H, I], BF16, **ikind).ap()
    wu_ap = nc.dram_tensor("wu", [H, I], BF16, **ikind).ap()
    wd_ap = nc.dram_tensor("wd", [I, H], BF16, **ikind).ap()
    wr_ap = nc.dram_tensor("wr", [H, E], BF16, **ikind).ap()
    if has_bias:
        bg_ap = nc.dram_tensor("bg", [P, IB], F32, **ikind).ap()
        bu_ap = nc.dram_tensor("bu", [P, IB], F32, **ikind).ap()
        bd_ap = nc.dram_tensor("bd", [P, H], F32, **ikind).ap()
    if bench:
        y_ap = nc.dram_tensor("y_int", [C, H], F32).ap()
        yext_ap = nc.dram_tensor("y", [P, 512], F32, kind="ExternalOutput").ap()
    else:
        y_ap = nc.dram_tensor("y", [C, H], F32, kind="ExternalOutput").ap()

    with tile.TileContext(nc) as tc:
        with (
            tc.tile_pool(name="weights", bufs=1) as wpool,
            tc.tile_pool(name="xgt", bufs=1) as xpool,
            tc.tile_pool(name="act", bufs=1) as apool,
            tc.tile_pool(name="router", bufs=2) as rpool,
            tc.tile_pool(name="elem", bufs=3) as epool,
            tc.tile_pool(name="yout", bufs=3) as ypool,
        ):
          wts = None
          for _rep in range(reps):
            ps_r_cm = tc.tile_pool(name="ps_r", bufs=2, space="PSUM")
            ps_r = ps_r_cm.__enter__()
            ps_g_cm = tc.tile_pool(name="ps_g", bufs=3, space="PSUM")
            ps_g = ps_g_cm.__enter__()
            ps_u_cm = tc.tile_pool(name="ps_u", bufs=3, space="PSUM")
            ps_u = ps_u_cm.__enter__()

            # ---- loads, in dependency order (all on the SP DMA queue):
            # transposed tokens + router weight + first part of the gate
            # weight unblock the router and layer-1 m=0; the rest follows.
            xgT = xpool.tile([P, HB, C], BF16)
            tchunks = [(0, 256)] + _chunks(C - 256, 512)
            tchunks = [(0, 256)] + [(o + 256, w) for (o, w) in tchunks[1:]]
            if wts is None:
                wr_sb = wpool.tile([P, HB, E], BF16)
                wg_sb = wpool.tile([P, HB, I], BF16)
                wu_sb = wpool.tile([P, HB, I], BF16)
                wd_sb = wpool.tile([P, IB, H], BF16)
                wg_r = wg_ap.rearrange("(ko p) i -> p ko i", p=P)
                wu_r = wu_ap.rearrange("(ko p) i -> p ko i", p=P)
                wd_r = wd_ap.rearrange("(ko p) i -> p ko i", p=P)
                for j, (n0, nw) in enumerate(tchunks):
                    nc.sync.dma_start_transpose(
                        xgT[:, :, n0:n0 + nw], xg_ap[n0:n0 + nw, :])
                    if j == 0:
                        # first slice of wg unblocks layer-1 m=0 as early as
                        # possible; router weight follows (router runs after
                        # layer 1 now)
                        nc.sync.dma_start(wg_sb[:, :, 0:256], wg_r[:, :, 0:256])
                        nc.sync.dma_start(
                            wr_sb[:], wr_ap.rearrange("(ko p) e -> p ko e", p=P))
                nc.sync.dma_start(wg_sb[:, :, 256:I], wg_r[:, :, 256:I])
                nc.sync.dma_start(wu_sb[:, :, 0:512], wu_r[:, :, 0:512])
                nc.sync.dma_start(wu_sb[:, :, 512:I], wu_r[:, :, 512:I])
                nc.sync.dma_start(wd_sb[:, :, 0:512], wd_r[:, :, 0:512])
                nc.sync.dma_start(wd_sb[:, :, 512:H], wd_r[:, :, 512:H])
                if has_bias:
                    bg_sb = wpool.tile([P, IB], F32)
                    nc.sync.dma_start(bg_sb[:], bg_ap[:, :])
                    bu_sb = wpool.tile([P, IB], F32)
                    nc.sync.dma_start(bu_sb[:], bu_ap[:, :])
                    bd_sb = wpool.tile([P, H], F32)
                    nc.sync.dma_start(bd_sb[:], bd_ap[:, :])
                else:
                    bg_sb = bu_sb = bd_sb = None
                if resident:
                    wts = (wr_sb, wg_sb, wu_sb, wd_sb, bg_sb, bu_sb, bd_sb)
            else:
                (wr_sb, wg_sb, wu_sb, wd_sb, bg_sb, bu_sb, bd_sb) = wts
                for j, (n0, nw) in enumerate(tchunks):
                    nc.sync.dma_start_transpose(
                        xgT[:, :, n0:n0 + nw], xg_ap[n0:n0 + nw, :])

            # ---- layer 1: gateT/upT [I-part, C] -> actT bf16 ----
            # loop (m, hb, n) so lhsT weights are reused across n-chunks
            actT = apool.tile([P, IB, C], BF16)
            nchunks = _chunks(C, 512)
            for m in range(IB):
                g_pss = [ps_g.tile([P, 512], F32, space="PSUM",
                                   tag="g_ps", name=f"g_ps{j}")[:, :nw]
                         for j, (n0, nw) in enumerate(nchunks)]
                for hb in range(HB):
                    for j, (n0, nw) in enumerate(nchunks):
                        nc.tensor.matmul(
                            g_pss[j],
                            lhsT=wg_sb[:, hb, m * P:(m + 1) * P],
                            rhs=xgT[:, hb, n0:n0 + nw],
                            start=(hb == 0), stop=(hb == HB - 1))
                u_pss = [ps_u.tile([P, 512], F32, space="PSUM",
                                   tag="u_ps", name=f"u_ps{j}")[:, :nw]
                         for j, (n0, nw) in enumerate(nchunks)]
                for hb in range(HB):
                    for j, (n0, nw) in enumerate(nchunks):
                        nc.tensor.matmul(
                            u_pss[j],
                            lhsT=wu_sb[:, hb, m * P:(m + 1) * P],
                            rhs=xgT[:, hb, n0:n0 + nw],
                            start=(hb == 0), stop=(hb == HB - 1))
                for j, (n0, nw) in enumerate(nchunks):
                    g_ps, u_ps = g_pss[j], u_pss[j]
                    gc = epool.tile([P, 512], BF16, tag="gc", name="gc")[:, :nw]
                    uc = epool.tile([P, 512], BF16, tag="uc", name="uc")[:, :nw]
                    sg = epool.tile([P, 512], BF16, tag="sg", name="sg")[:, :nw]
                    if has_bias:
                        nc.vector.tensor_add(
                            gc, g_ps, bg_sb[:, m:m + 1].to_broadcast([P, nw]))
                        nc.vector.tensor_scalar_min(gc, gc, LIMIT)
                        nc.vector.tensor_add(
                            uc, u_ps, bu_sb[:, m:m + 1].to_broadcast([P, nw]))
                        nc.vector.tensor_scalar(
                            uc, uc, LIMIT, -LIMIT,
                            mybir.AluOpType.min, mybir.AluOpType.max)
                    else:
                        # gate = min(gate, LIMIT); up = clip(up, -LIMIT, LIMIT)
                        nc.vector.tensor_scalar_min(gc, g_ps, LIMIT)
                        nc.vector.tensor_scalar(
                            uc, u_ps, LIMIT, -LIMIT,
                            mybir.AluOpType.min, mybir.AluOpType.max)
                    # sg = sigmoid(alpha * gate)   (ACT)
                    nc.scalar.activation(sg, gc,
                                         mybir.ActivationFunctionType.Sigmoid,
                                         scale=ALPHA)
                    # up1 = up + 1   (ACT, Copy applies in*scale + bias)
                    up1 = epool.tile([P, 512], BF16, tag="up1", name="up1")[:, :nw]
                    nc.scalar.activation(up1, uc,
                                         mybir.ActivationFunctionType.Copy,
                                         bias=1.0)
                    # glu = gate * sg ; act = up1 * glu   (DVE, bf16 fast mode)
                    nc.vector.tensor_mul(gc, gc, sg)
                    nc.vector.tensor_mul(actT[:, m, n0:n0 + nw], up1, gc)

            # ---- router: logits[c-part, e] per 128-token block, batched math ----
            w_sb = rpool.tile([P, CB], F32, tag="wslot")  # routing weight per slot
            if no_router:  # ablation for benchmarking only
                nc.vector.memset(w_sb[:], 0.5)
            else:
              lg_all = rpool.tile([P, CB, E], F32, tag="lg_all")
              mx_all = rpool.tile([P, CB, 8], F32, tag="mx_all")
              rs = rpool.tile([P, 4, CB], F32, tag="rscratch")
              for cb in range(CB):
                  ps_l = ps_r.tile([P, E], F32, space="PSUM", tag="ps_l", name="ps_l")
                  for hb in range(HB):
                      nc.tensor.matmul(
                          ps_l,
                          lhsT=xgT[:, hb, cb * P:(cb + 1) * P],
                          rhs=wr_sb[:, hb, :],
                          start=(hb == 0), stop=(hb == HB - 1),
                      )
                  nc.scalar.activation(lg_all[:, cb, :], ps_l[:],
                                       mybir.ActivationFunctionType.Copy)
                  nc.vector.max(mx_all[:, cb, :], lg_all[:, cb, :])
              m1 = mx_all[:, :, 0]   # [P, CB] strided views
              m2 = mx_all[:, :, 1]
              l0 = lg_all[:, :, 0]
              d2, d0, rec = rs[:, 0, :], rs[:, 1, :], rs[:, 2, :]
              nc.vector.tensor_sub(d2, m2, m1)
              nc.scalar.activation(d2, d2, mybir.ActivationFunctionType.Exp)
              nc.vector.tensor_scalar_add(d2, d2, 1.0)
              nc.vector.reciprocal(rec, d2)
              nc.vector.tensor_sub(d0, l0, m1)
              nc.scalar.activation(d0, d0, mybir.ActivationFunctionType.Exp)
              nc.vector.tensor_mul(w_sb[:], d0, rec)

            # ---- layer 2: y[c-part, H] = actT.T @ wd, scaled by w ----
            # loop (cb, ib, n) so lhsT actT blocks are reused across n-chunks
            hchunks = _chunks(H, 512)
            ps_u_cm.__exit__(None, None, None)
            ps_g_cm.__exit__(None, None, None)
            ps_r_cm.__exit__(None, None, None)
            ps_y2_cm = tc.tile_pool(name="ps_y2", bufs=4, space="PSUM")
            ps_y2 = ps_y2_cm.__enter__()
            for cb in range(CB):
                y_pss = [ps_y2.tile([P, 512], F32, space="PSUM",
                                    tag="y_ps", name=f"y_ps{j}")[:, :nw]
                         for j, (n0, nw) in enumerate(hchunks)]
                for ib in range(IB):
                    for j, (n0, nw) in enumerate(hchunks):
                        nc.tensor.matmul(
                            y_pss[j],
                            lhsT=actT[:, ib, cb * P:(cb + 1) * P],
                            rhs=wd_sb[:, ib, n0:n0 + nw],
                            start=(ib == 0), stop=(ib == IB - 1))
                y_sb = ypool.tile([P, H], F32, tag="ysb", name="y_sb")
                for j, (n0, nw) in enumerate(hchunks):
                    if has_bias:
                        nc.vector.tensor_add(y_sb[:, n0:n0 + nw], y_pss[j],
                                             bd_sb[:, n0:n0 + nw])
                        nc.vector.tensor_mul(
                            y_sb[:, n0:n0 + nw], y_sb[:, n0:n0 + nw],
                            w_sb[:, cb:cb + 1].to_broadcast([P, nw]))
                    else:
                        # y = y_ps * w  on ACT: Copy(in * scale), scale is a
                        # per-partition AP
                        nc.scalar.activation(
                            y_sb[:, n0:n0 + nw], y_pss[j],
                            mybir.ActivationFunctionType.Copy,
                            scale=w_sb[:, cb:cb + 1])
                nc.sync.dma_start(y_ap[cb * P:(cb + 1) * P, :], y_sb[:])
            ps_y2_cm.__exit__(None, None, None)
            if bench:
                ylast = ypool.tile([P, 512], F32, tag="ysb", name="ylast")
                nc.sync.dma_start(ylast[:], y_ap[0:P, 0:512])
                nc.sync.dma_start(yext_ap[:, :], ylast[:])

    nc.compile()
    return nc


_KERNEL_CACHE: dict = {}


def build_expert_kernel_replicated(C: int, has_bias: bool, reps: int):
    return build_expert_kernel(C, has_bias, reps, bench=True)


def _get_kernel(C: int, has_bias: bool):
    key = (C, has_bias)
    if key not in _KERNEL_CACHE:
        _KERNEL_CACHE[key] = build_expert_kernel(C, has_bias)
    return _KERNEL_CACHE[key]


def _route(x, router_weight):
    """Host-side top-2 routing decision (indices only; weights computed on
    device). Mirrors jax.lax.top_k tie-breaking (first index wins)."""
    logits = x @ router_weight  # [T, E] f32
    # top-2 indices; argsort of -logits is stable so equals top_k on ties
    top2 = np.argsort(-logits, axis=1, kind="stable")[:, :K]
    return top2


def prepare_in_maps(hidden_states, router_weight, gate_up_proj,
                    gate_up_proj_bias, down_proj, down_proj_bias):
    x = np.ascontiguousarray(
        np.asarray(hidden_states, dtype=np.float32).reshape(T, H))
    rw = np.asarray(router_weight, dtype=np.float32)
    top2 = _route(x, rw)

    idx_lists = []
    for c in range(N_CORES):
        sel = np.nonzero((top2 == c).any(axis=1))[0]
        idx_lists.append(sel.astype(np.int64))
    max_load = max(len(s) for s in idx_lists)
    C = max(_ceil_to(max_load, P), 512)

    xbf = x.astype(NP_BF16)
    gup = np.asarray(gate_up_proj, dtype=np.float32)
    gub = np.asarray(gate_up_proj_bias, dtype=np.float32)
    dwn = np.asarray(down_proj, dtype=np.float32)
    dwb = np.asarray(down_proj_bias, dtype=np.float32)
    has_bias = bool(np.any(gub) or np.any(dwb))

    in_maps = []
    for c in range(N_CORES):
        idx = idx_lists[c]
        xg = np.zeros((C, H), dtype=NP_BF16)
        xg[:len(idx)] = xbf[idx]
        perm = [c] + [e for e in range(E) if e != c]
        m = {
            "xg": xg,
            "wg": np.ascontiguousarray(gup[c, :, 0::2]).astype(NP_BF16),
            "wu": np.ascontiguousarray(gup[c, :, 1::2]).astype(NP_BF16),
            "wd": np.ascontiguousarray(dwn[c]).astype(NP_BF16),
            "wr": np.ascontiguousarray(rw[:, perm]).astype(NP_BF16),
        }
        if has_bias:
            m["bg"] = np.ascontiguousarray(
                gub[c, 0::2].reshape(IB, P).T).astype(np.float32)
            m["bu"] = np.ascontiguousarray(
                gub[c, 1::2].reshape(IB, P).T).astype(np.float32)
            m["bd"] = np.broadcast_to(dwb[c], (P, H)).copy().astype(np.float32)
        in_maps.append(m)
    return in_maps, idx_lists, C, has_bias


def combine(results, idx_lists):
    out = np.zeros((T, H), dtype=np.float32)
    for c in range(N_CORES):
        idx = idx_lists[c]
        out[idx] += results[c]["y"][:len(idx)]
    return out.reshape(B, S, H)


def kernel(hidden_states, router_weight, gate_up_proj, gate_up_proj_bias,
           down_proj, down_proj_bias):
    in_maps, idx_lists, C, has_bias = prepare_in_maps(
        hidden_states, router_weight, gate_up_proj, gate_up_proj_bias,
        down_proj, down_proj_bias)
    nc = _get_kernel(C, has_bias)
    last_err = None
    for attempt in range(3):
        try:
            res = run_bass_kernel_spmd(nc, in_maps,
                                       core_ids=list(range(N_CORES)))
            break
        except Exception as e:  # transient device/runtime hiccups
            last_err = e
            if attempt == 2:
                raise
            time.sleep(5 * (attempt + 1))
    return combine(res.results, idx_lists)

